# revision 1
# baseline (speedup 1.0000x reference)
"""BPLoss Trainium2 kernel: 8-core SPMD over the detection (N) axis.

Per core (shard of R=12544 rows, padded):
  - gather gt table rows per detection via one-hot matmul on TensorE
  - masked row-max of class_scores: gpsimd builds corr = -BIG*onehot(label),
    the class-score tile is DMA-accumulated (SWDGE accum_op=add) onto corr,
    VectorE does a single fused reduce_max pass
  - ScalarE computes log of the row maxes
  - fused multiply-accumulate reductions produce per-partition partial sums
Host: shard/pad inputs, sum the 8x[128,2] partials, combine.
"""
import numpy as np
import concourse.bass as bass
import concourse.tile as tile
from concourse import bacc, mybir
from concourse.bass_utils import run_bass_kernel_spmd

N, C, M = 100000, 1024, 128
NCORES = 8
T = 98              # 128-row tiles per core
R = T * 128         # 12544 rows per core
BIG = 1024.0

f32 = mybir.dt.float32
i32 = mybir.dt.int32
OP = mybir.AluOpType
AF = mybir.ActivationFunctionType
AX = mybir.AxisListType

# tuning knobs
CS_BUFS = 6
SMALL_BUFS = 8
PSUM_BUFS = 3


def build_nc():
    nc = bacc.Bacc("TRN2", target_bir_lowering=False, debug=False, num_devices=NCORES)
    cs = nc.dram_tensor("cs", [T, 128, C], f32, kind="ExternalInput").ap()
    idx = nc.dram_tensor("idx", [T, 128, 1], i32, kind="ExternalInput").ap()
    z_pt = nc.dram_tensor("z_pt", [128, T], f32, kind="ExternalInput").ap()
    r_pt = nc.dram_tensor("r_pt", [128, T], f32, kind="ExternalInput").ap()
    xywh_pt = nc.dram_tensor("xywh_pt", [128, T * 4], f32, kind="ExternalInput").ap()
    tbl = nc.dram_tensor("tbl", [M, 5], f32, kind="ExternalInput").ap()
    out = nc.dram_tensor("out", [128, 2], f32, kind="ExternalOutput").ap()

    with tile.TileContext(nc) as tc:
        with (
            tc.tile_pool(name="const", bufs=1) as constp,
            tc.tile_pool(name="csp", bufs=CS_BUFS) as csp,
            tc.tile_pool(name="small", bufs=SMALL_BUFS) as smallp,
            tc.tile_pool(name="psA", bufs=PSUM_BUFS, space="PSUM") as psA,
            tc.tile_pool(name="psB", bufs=PSUM_BUFS, space="PSUM") as psB,
        ):
            # ---- constants ----
            iota_c_i = constp.tile([128, C], i32)
            nc.gpsimd.iota(iota_c_i[:], pattern=[[1, C]], base=0, channel_multiplier=0)
            iota_c = constp.tile([128, C], f32)
            nc.vector.tensor_copy(iota_c[:], iota_c_i[:])
            iota_m_i = constp.tile([128, M], i32)
            nc.gpsimd.iota(iota_m_i[:], pattern=[[1, M]], base=0, channel_multiplier=0)
            iota_m = constp.tile([128, M], f32)
            nc.vector.tensor_copy(iota_m[:], iota_m_i[:])
            iota_p_i = constp.tile([128, 1], i32)
            nc.gpsimd.iota(iota_p_i[:], pattern=[[0, 1]], base=0, channel_multiplier=1)
            iota_p = constp.tile([128, 1], f32)
            nc.vector.tensor_copy(iota_p[:], iota_p_i[:])
            ident = constp.tile([128, 128], f32)
            nc.vector.tensor_scalar(
                out=ident[:], in0=iota_m[:], scalar1=iota_p[:], scalar2=None,
                op0=OP.is_equal,
            )
            tbl_sb = constp.tile([M, 5], f32)
            nc.sync.dma_start(out=tbl_sb[:], in_=tbl[:])
            z_sb = constp.tile([128, T], f32)
            nc.sync.dma_start(out=z_sb[:], in_=z_pt[:])
            r_sb = constp.tile([128, T], f32)
            nc.sync.dma_start(out=r_sb[:], in_=r_pt[:])
            w_sb = constp.tile([128, T], f32)
            nc.vector.tensor_add(w_sb[:], z_sb[:], r_sb[:])
            xywh_sb = constp.tile([128, T, 4], f32)
            nc.sync.dma_start(
                out=xywh_sb[:], in_=xywh_pt[:].rearrange("p (t c) -> p t c", c=4)
            )

            G_all = constp.tile([128, T, 4], f32)   # gathered gt_xywh (ACT-only writes)
            rowmax = constp.tile([128, T], f32)
            out_sb = constp.tile([128, 2], f32)

            # ---- main loop over 128-row tiles ----
            for t in range(T):
                idxt = smallp.tile([128, 1], i32)
                nc.sync.dma_start(out=idxt[:], in_=idx[t])
                idxf = smallp.tile([128, 1], f32)
                nc.vector.tensor_copy(idxf[:], idxt[:])
                # one-hot O[row, m] = (m == idx_row)
                O = smallp.tile([128, M], f32)
                nc.vector.tensor_scalar(
                    out=O[:], in0=iota_m[:], scalar1=idxf[:], scalar2=None,
                    op0=OP.is_equal,
                )
                OT_ps = psA.tile([128, 128], f32)
                nc.tensor.transpose(OT_ps[:], O[:], ident[:])
                OT_sb = smallp.tile([128, 128], f32)
                nc.scalar.copy(out=OT_sb[:], in_=OT_ps[:])
                # G[row, :] = tbl[idx_row, :]
                G_ps = psB.tile([128, 5], f32)
                nc.tensor.matmul(G_ps[:], OT_sb[:], tbl_sb[:], start=True, stop=True)
                Gt = smallp.tile([128, 5], f32)
                nc.scalar.copy(out=Gt[:], in_=G_ps[:])
                nc.scalar.copy(out=G_all[:, t, :], in_=G_ps[:, 0:4])
                # corr = -BIG * onehot_C(label)
                corr = csp.tile([128, C], f32)
                nc.gpsimd.tensor_scalar(
                    out=corr[:], in0=iota_c[:], scalar1=Gt[:, 4:5], scalar2=-BIG,
                    op0=OP.is_equal, op1=OP.mult,
                )
                # masked scores materialize via accumulate-DMA
                nc.gpsimd.dma_start(out=corr[:], in_=cs[t], accum_op=OP.add)
                nc.vector.reduce_max(rowmax[:, t : t + 1], corr[:], axis=AX.X)

            # ---- epilogue ----
            lm = constp.tile([128, T], f32)
            nc.scalar.activation(out=lm[:], in_=rowmax[:], func=AF.Ln)
            scr1 = constp.tile([128, T], f32)
            nc.vector.scalar_tensor_tensor(
                out=scr1[:], in0=w_sb[:], scalar=0.0, in1=lm[:],
                op0=OP.bypass, op1=OP.mult, accum_out=out_sb[:, 0:1],
            )
            diff = constp.tile([128, T, 4], f32)
            nc.vector.tensor_sub(diff[:], xywh_sb[:], G_all[:])
            sq = constp.tile([128, T, 4], f32)
            nc.vector.tensor_mul(sq[:], diff[:], diff[:])
            d_pt = constp.tile([128, T], f32)
            nc.vector.reduce_sum(d_pt[:], sq[:], axis=AX.X)
            scr2 = constp.tile([128, T], f32)
            nc.vector.scalar_tensor_tensor(
                out=scr2[:], in0=z_sb[:], scalar=0.0, in1=d_pt[:],
                op0=OP.bypass, op1=OP.mult, accum_out=out_sb[:, 1:2],
            )
            nc.sync.dma_start(out=out[:], in_=out_sb[:])

    nc.compile()
    return nc


def make_in_maps(class_scores, xywh, z, r, nearest_gt_idx, gt_class_labels, gt_xywh):
    cs = np.ascontiguousarray(np.asarray(class_scores, dtype=np.float32))
    xywh = np.ascontiguousarray(np.asarray(xywh, dtype=np.float32))
    z = np.ascontiguousarray(np.asarray(z, dtype=np.float32))
    r = np.ascontiguousarray(np.asarray(r, dtype=np.float32))
    idx = np.ascontiguousarray(np.asarray(nearest_gt_idx).astype(np.int32))
    gtl = np.asarray(gt_class_labels).astype(np.float32)
    gtx = np.asarray(gt_xywh, dtype=np.float32)

    tbl = np.concatenate([gtx, gtl[:, None]], axis=1).astype(np.float32)
    tbl = np.ascontiguousarray(tbl)

    in_maps = []
    for c in range(NCORES):
        lo, hi = c * R, (c + 1) * R
        if hi <= N:
            cs_s, xywh_s, z_s, r_s, idx_s = (
                cs[lo:hi], xywh[lo:hi], z[lo:hi], r[lo:hi], idx[lo:hi],
            )
        else:
            n_real = N - lo
            cs_s = np.ones((R, C), dtype=np.float32)
            cs_s[:n_real] = cs[lo:]
            xywh_s = np.zeros((R, 4), dtype=np.float32)
            xywh_s[:n_real] = xywh[lo:]
            z_s = np.zeros(R, dtype=np.float32)
            z_s[:n_real] = z[lo:]
            r_s = np.zeros(R, dtype=np.float32)
            r_s[:n_real] = r[lo:]
            idx_s = np.zeros(R, dtype=np.int32)
            idx_s[:n_real] = idx[lo:]
        in_maps.append({
            "cs": cs_s.reshape(T, 128, C),
            "idx": np.ascontiguousarray(idx_s.reshape(T, 128, 1)),
            "z_pt": np.ascontiguousarray(z_s.reshape(T, 128).T),
            "r_pt": np.ascontiguousarray(r_s.reshape(T, 128).T),
            "xywh_pt": np.ascontiguousarray(
                xywh_s.reshape(T, 128, 4).transpose(1, 0, 2).reshape(128, T * 4)
            ),
            "tbl": tbl,
        })
    return in_maps


def combine_outputs(outs):
    """outs: list of [128, 2] per-core partials -> final [1] float32."""
    partA = float(sum(o[:, 0].astype(np.float64).sum() for o in outs))
    partB = float(sum(o[:, 1].astype(np.float64).sum() for o in outs))
    with np.errstate(over="ignore", under="ignore"):
        tps = np.exp(-partB)
    val = -partA + tps
    return np.array([val], dtype=np.float32)


_NC_CACHE = None


def get_nc():
    global _NC_CACHE
    if _NC_CACHE is None:
        _NC_CACHE = build_nc()
    return _NC_CACHE


def kernel(**inputs) -> np.ndarray:
    nc = get_nc()
    in_maps = make_in_maps(**inputs)
    res = run_bass_kernel_spmd(nc, in_maps, core_ids=list(range(NCORES)))
    return combine_outputs([res.results[c]["out"] for c in range(NCORES)])
